# revision 1
# baseline (speedup 1.0000x reference)
"""Trainium2 Bass kernel for nn_AttentionBlock (sparse attention block).

Reference computation (B=4, C=512, T=2048, H=8 heads, 32 GN groups):
    xn  = GroupNorm(x) * gn_w + gn_b
    qkv = qkv_w @ xn + qkv_b            (1x1 conv)
    q,k,v = split(reshape(qkv, [B*H, 192, T])) ; each += pos
    S   = (q*s)^T (k*s),  s = ch^-0.25  => scale 1/8 on logits
    S[mask keys] = -1e9 ; P = softmax(S, axis=keys)
    h   = P @ v ; out = x + proj_w @ h + proj_b

Mask quirk (faithful to the reference): jnp.tile(mask,(H,1,1)) tiles
head-major, so attention row n = b*H + h uses mask[n % B] = mask[h % 4] —
every batch's head h is masked with mask[h mod 4], not its own batch mask.

Sharding: 8 cores = (batch b, query-half j).  Each core computes
out[b][:, j*1024:(j+1)*1024] completely; host concatenates.  No collectives.

Sparsity: host compacts the key axis per mask-group m = h%4 with
keep_m = ~mask[m] (about half of T), padded to a common multiple of 128.
Padded key rows get an exp-bias of -1e9 so they contribute exactly 0.

Head layout on device: slot order [0,4,1,5,2,6,3,7] so the two heads of a
mask-group (m, m+4) sit in one 128-partition pair; host reorders the qkv
weights / biases / pos / proj rows to match, so the device never permutes.

Device layout tricks: scores are computed transposed, S^T [keys, queries]:
  - the pad bias is per-partition and folds into the ACT exp for free,
  - the softmax denominator comes from an extra ones-column appended to V^T
    during the PV matmul (row 64 of the PV psum accumulates sum_s exp(S)).
GroupNorm statistics are folded on the host into a per-channel affine (A, B)
so the device applies xn = x*A + B with one tensor_scalar op per tile.
"""

import numpy as np
import ml_dtypes

B, C, T, H = 4, 512, 2048, 8
CH = C // H          # 64 channels per head
TH = T // 2          # 1024 query columns per core
P = 128
NUM_GROUPS = 32
GS = C // NUM_GROUPS  # 16 channels per group
EPS = 1e-5
BF16 = ml_dtypes.bfloat16
NMG = 4              # mask groups (= B); group m covers heads m and m+4
PERM = [0, 4, 1, 5, 2, 6, 3, 7]  # slot s holds true head PERM[s]

_graph_cache = {}


def _build(nkv):
    """Build the Bass graph for one core (SPMD: all 8 cores run this graph)."""
    import concourse.tile as tile
    from concourse import bacc, mybir

    f32 = mybir.dt.float32
    bf16 = mybir.dt.bfloat16
    AF = mybir.ActivationFunctionType
    OP = mybir.AluOpType

    sc_n = nkv // P  # number of 128-wide key chunks

    nc = bacc.Bacc("TRN2")

    # ---- DRAM parameters (per-core shards; host fills these) ----
    d_xq = nc.dram_tensor("x_q", [C, TH], bf16, kind="ExternalInput")
    d_xkv = nc.dram_tensor("x_kv", [NMG, C, nkv], bf16, kind="ExternalInput")
    d_xres = nc.dram_tensor("x_res", [C, TH], f32, kind="ExternalInput")
    d_AB = nc.dram_tensor("gn_AB", [C, 2], f32, kind="ExternalInput")
    d_posq = nc.dram_tensor("pos_q", [C, TH], bf16, kind="ExternalInput")
    d_poskv = nc.dram_tensor("pos_kv", [NMG, P, nkv], bf16, kind="ExternalInput")
    d_posT = nc.dram_tensor("posT_kv", [NMG, nkv, 130], bf16, kind="ExternalInput")
    d_wqkvT = nc.dram_tensor("wqkvT", [C, 3 * C], bf16, kind="ExternalInput")
    d_wpT = nc.dram_tensor("wpT", [C, C], bf16, kind="ExternalInput")
    d_pad = nc.dram_tensor("pad_bias", [NMG, nkv, 1], f32, kind="ExternalInput")
    d_out = nc.dram_tensor("out", [C, TH], f32, kind="ExternalOutput")

    with tile.TileContext(nc) as tc, \
         tc.tile_pool(name="persist", bufs=1) as pers:

        def ptile(shape, dt_, name):
            return pers.tile(shape, dt_, tag=name, name=name)

        # --- tiny exp to pull the ACT table load off the critical path ---
        warm_in = ptile([1, 1], f32, "warm_in")
        warm_out = ptile([1, 1], f32, "warm_out")
        nc.vector.memset(warm_in, 0.0)
        nc.scalar.activation(out=warm_out, in_=warm_in, func=AF.Exp)

        # --- persistent SBUF arrays ---
        xq = [ptile([P, TH], bf16, f"xq{i}") for i in range(4)]
        xkv = [[ptile([P, nkv], bf16, f"xkv{m}_{i}") for i in range(4)]
               for m in range(NMG)]
        gnAB = [ptile([P, 2], f32, f"gnAB{i}") for i in range(4)]
        wq = [ptile([P, 3 * C], bf16, f"wq{i}") for i in range(4)]
        wp = [ptile([CH, C], bf16, f"wp{i}") for i in range(8)]
        posq = [ptile([P, TH], bf16, f"posq{i}") for i in range(4)]
        poskv = [ptile([P, nkv], bf16, f"poskv{m}") for m in range(NMG)]
        q_sb = [ptile([P, TH], bf16, f"q{i}") for i in range(4)]
        k_sb = [ptile([P, nkv], bf16, f"k{m}") for m in range(NMG)]
        vhat = [[ptile([P, 130], bf16, f"vhat{m}_{s}") for s in range(sc_n)]
                for m in range(NMG)]
        # per-head-slot attention output (all at base partition 0)
        h_sb = [ptile([CH, TH], bf16, f"h{s}") for s in range(H)]
        xres = [ptile([P, TH], f32, f"xres{i}") for i in range(4)]
        pad_sb = [[ptile([P, 1], f32, f"pad{m}_{s}") for s in range(sc_n)]
                  for m in range(NMG)]
        ones_t = ptile([65, CH], f32, "ones_t")
        nc.vector.memset(ones_t, 1.0)

        # --- input DMAs (emitted early; HW-DGE prefetches) ---
        for i in range(4):
            r = slice(i * P, (i + 1) * P)
            nc.sync.dma_start(gnAB[i], d_AB[r, :])
            for m in range(NMG):
                nc.sync.dma_start(xkv[m][i], d_xkv[m, r, :])
            nc.sync.dma_start(wq[i], d_wqkvT[r, :])
        for m in range(NMG):
            for s in range(sc_n):
                r = slice(s * P, (s + 1) * P)
                nc.sync.dma_start(vhat[m][s], d_posT[m, r, :])
        for i in range(4):
            r = slice(i * P, (i + 1) * P)
            nc.sync.dma_start(xq[i], d_xq[r, :])
            nc.sync.dma_start(posq[i], d_posq[r, :])
        for m in range(NMG):
            nc.sync.dma_start(poskv[m], d_poskv[m, :, :])
            for s in range(sc_n):
                r = slice(s * P, (s + 1) * P)
                nc.sync.dma_start(pad_sb[m][s], d_pad[m, r, :])
        for i in range(4):
            nc.sync.dma_start(xres[i], d_xres[i * P:(i + 1) * P, :])
        for cc in range(8):
            nc.sync.dma_start(wp[cc], d_wpT[cc * CH:(cc + 1) * CH, :])

        # --- GroupNorm as per-channel affine (host-computed A, B) ---
        for i in range(4):
            nc.vector.tensor_scalar(
                out=xq[i], in0=xq[i], scalar1=gnAB[i][:, 0:1],
                scalar2=gnAB[i][:, 1:2], op0=OP.mult, op1=OP.add)
            for m in range(NMG):
                nc.vector.tensor_scalar(
                    out=xkv[m][i], in0=xkv[m][i], scalar1=gnAB[i][:, 0:1],
                    scalar2=gnAB[i][:, 1:2], op0=OP.mult, op1=OP.add)

        with tc.tile_pool(name="mm", bufs=2, space="PSUM") as mmp, \
             tc.tile_pool(name="opool", bufs=2, space="PSUM") as opl, \
             tc.tile_pool(name="exps", bufs=6) as epl, \
             tc.tile_pool(name="misc", bufs=2) as msc:

            def emit_v(m):
                # v^T for group m: (xn_kv_m)^T @ w_v[group m slots]^T
                for s in range(sc_n):
                    pv = mmp.tile([P, P], f32, tag="mm", name=f"psv{m}_{s}")
                    for i in range(4):
                        nc.tensor.matmul(
                            pv, xkv[m][i][:, s * P:(s + 1) * P],
                            wq[i][:, 2 * C + m * P:2 * C + (m + 1) * P],
                            start=(i == 0), stop=(i == 3))
                    vh_view = vhat[m][s].rearrange(
                        "p (h c) -> p h c", c=65)[:, :, 0:CH]
                    ps_view = pv.rearrange("p (h c) -> p h c", c=CH)
                    nc.vector.tensor_tensor(vh_view, ps_view, vh_view, OP.add)

            nb_blocks = [(st, min(512, nkv - st)) for st in range(0, nkv, 512)]

            def emit_qk(m):
                # q channels (slot order) [128*m, 128*m+128)
                pq = mmp.tile([P, TH], f32, tag="mm", name=f"psq{m}")
                for tb in range(2):
                    for i in range(4):
                        nc.tensor.matmul(
                            pq[:, tb * 512:(tb + 1) * 512],
                            wq[i][:, m * P:(m + 1) * P],
                            xq[i][:, tb * 512:(tb + 1) * 512],
                            start=(i == 0), stop=(i == 3))
                nc.vector.tensor_add(q_sb[m], pq, posq[m])
                for bi, (st, w) in enumerate(nb_blocks):
                    pk = mmp.tile([P, 512], f32, tag="mm", name=f"psk{m}_{bi}")
                    for i in range(4):
                        nc.tensor.matmul(
                            pk[:, 0:w],
                            wq[i][:, C + m * P:C + (m + 1) * P],
                            xkv[m][i][:, st:st + w],
                            start=(i == 0), stop=(i == 3))
                    nc.vector.tensor_add(
                        k_sb[m][:, st:st + w], pk[:, 0:w],
                        poskv[m][:, st:st + w])

            def emit_attention(m):
                # pair m: head slot a=2m (partitions 0:64), b=2m+1 (64:128)
                o_a = opl.tile([65, TH], f32, tag="O", name=f"oa{m}")
                o_b = opl.tile([65, TH], f32, tag="O", name=f"ob{m}")
                for s in range(sc_n):
                    sa = mmp.tile([P, TH], f32, tag="mm", name=f"sa{m}_{s}")
                    sb = mmp.tile([P, TH], f32, tag="mm", name=f"sb{m}_{s}")
                    for tb in range(2):
                        nc.tensor.matmul(
                            sa[:, tb * 512:(tb + 1) * 512],
                            k_sb[m][0:64, s * P:(s + 1) * P],
                            q_sb[m][0:64, tb * 512:(tb + 1) * 512],
                            start=True, stop=True)
                    for tb in range(2):
                        nc.tensor.matmul(
                            sb[:, tb * 512:(tb + 1) * 512],
                            k_sb[m][64:128, s * P:(s + 1) * P],
                            q_sb[m][64:128, tb * 512:(tb + 1) * 512],
                            start=True, stop=True, tile_position=(64, 0))
                    ex = epl.tile([P, 2 * TH], bf16, tag="expS", name=f"ex{m}_{s}")
                    nc.scalar.activation(
                        out=ex[:, 0:TH], in_=sa, func=AF.Exp,
                        bias=pad_sb[m][s], scale=0.125)
                    nc.scalar.activation(
                        out=ex[:, TH:2 * TH], in_=sb, func=AF.Exp,
                        bias=pad_sb[m][s], scale=0.125)
                    for tb in range(2):
                        nc.tensor.matmul(
                            o_a[:, tb * 512:(tb + 1) * 512],
                            vhat[m][s][:, 0:65],
                            ex[:, tb * 512:(tb + 1) * 512],
                            start=(s == 0), stop=(s == sc_n - 1))
                    for tb in range(2):
                        nc.tensor.matmul(
                            o_b[:, tb * 512:(tb + 1) * 512],
                            vhat[m][s][:, 65:130],
                            ex[:, TH + tb * 512:TH + (tb + 1) * 512],
                            start=(s == 0), stop=(s == sc_n - 1))
                return o_a, o_b

            def emit_normalize(m, o_a, o_b):
                # normalize: h = O[0:64] / l, l = O[64].  The reciprocal and
                # the ones-lhsT live on partition 64 so matmul operand pairs
                # share a base partition; bc lands on partitions 0:64.
                rc = msc.tile([65, 2 * TH], f32, tag="recip", name=f"rc{m}")
                nc.vector.reciprocal(rc[64:65, 0:TH], o_a[64:65, :])
                nc.vector.reciprocal(rc[64:65, TH:2 * TH], o_b[64:65, :])
                bc_a = mmp.tile([CH, TH], f32, tag="mm", name=f"bca{m}")
                bc_b = mmp.tile([CH, TH], f32, tag="mm", name=f"bcb{m}")
                for tb in range(2):
                    nc.tensor.matmul(
                        bc_a[:, tb * 512:(tb + 1) * 512],
                        ones_t[64:65, :],
                        rc[64:65, tb * 512:(tb + 1) * 512],
                        start=True, stop=True, tile_position=(64, 0))
                    nc.tensor.matmul(
                        bc_b[:, tb * 512:(tb + 1) * 512],
                        ones_t[64:65, :],
                        rc[64:65, TH + tb * 512:TH + (tb + 1) * 512],
                        start=True, stop=True, tile_position=(64, 0))
                # DVE reads at most one PSUM operand per op: stage bc in SBUF
                bc_sb = msc.tile([CH, 2 * TH], bf16, tag="bcsb", name=f"bcs{m}")
                nc.vector.tensor_copy(out=bc_sb[:, 0:TH], in_=bc_a)
                nc.vector.tensor_copy(out=bc_sb[:, TH:2 * TH], in_=bc_b)
                nc.vector.tensor_mul(
                    h_sb[2 * m], o_a[0:64, :], bc_sb[:, 0:TH])
                nc.vector.tensor_mul(
                    h_sb[2 * m + 1], o_b[0:64, :], bc_sb[:, TH:2 * TH])

            # interleave: emit next group's qk before normalizing the
            # previous group so PE/ACT stay fed during the slow reciprocal
            for m in range(NMG):
                emit_v(m)
            pending = {}
            for m in range(NMG):
                emit_qk(m)
                if m - 1 in pending:
                    emit_normalize(m - 1, *pending.pop(m - 1))
                pending[m] = emit_attention(m)
            emit_normalize(NMG - 1, *pending.pop(NMG - 1))

            # ---- proj + residual (contraction in 8 chunks of 64) ----
            for ci in range(4):
                pp = mmp.tile([P, TH], f32, tag="mm", name=f"pp{ci}")
                for tb in range(2):
                    for cc in range(8):
                        nc.tensor.matmul(
                            pp[:, tb * 512:(tb + 1) * 512],
                            wp[cc][:, ci * P:(ci + 1) * P],
                            h_sb[cc][:, tb * 512:(tb + 1) * 512],
                            start=(cc == 0), stop=(cc == 7))
                ot = msc.tile([P, TH], f32, tag="out", name=f"ot{ci}")
                nc.vector.tensor_add(ot, pp, xres[ci])
                nc.sync.dma_start(d_out[ci * P:(ci + 1) * P, :], ot)

    nc.finalize()
    return nc


def _prepare(inputs):
    """Host-side shard preparation. Returns (nkv, in_maps)."""
    x = np.asarray(inputs["x"], dtype=np.float32)
    pos = np.asarray(inputs["pos"], dtype=np.float32)
    mask = np.asarray(inputs["mask"])
    gn_w = np.asarray(inputs["gn_w"], dtype=np.float32)
    gn_b = np.asarray(inputs["gn_b"], dtype=np.float32)
    qkv_w = np.asarray(inputs["qkv_w"], dtype=np.float32)
    qkv_b = np.asarray(inputs["qkv_b"], dtype=np.float32)
    proj_w = np.asarray(inputs["proj_w"], dtype=np.float32)
    proj_b = np.asarray(inputs["proj_b"], dtype=np.float32)

    # GroupNorm folded to per-channel affine per batch (stats over full T,
    # matching the reference exactly).
    xg = x.reshape(B, NUM_GROUPS, GS, T)
    mu = xg.mean(axis=(2, 3))
    var = xg.var(axis=(2, 3))
    rs = 1.0 / np.sqrt(var + EPS)
    rs_c = np.repeat(rs, GS, axis=1)
    mu_c = np.repeat(mu, GS, axis=1)
    A_all = rs_c * gn_w[None, :]
    B_all = gn_b[None, :] - mu_c * A_all

    # reorder qkv weights: reference splits rows as [h, (q|k|v), 64]; we
    # additionally permute heads into slot order PERM.
    perm = np.asarray(PERM)
    w3 = qkv_w.reshape(H, 3, CH, C)
    b3 = qkv_b.reshape(H, 3, CH)
    wq_r = w3[perm, 0].reshape(C, C)
    wk_r = w3[perm, 1].reshape(C, C)
    wv_r = w3[perm, 2].reshape(C, C)
    bq = b3[perm, 0].reshape(C)
    bk = b3[perm, 1].reshape(C)
    bv = b3[perm, 2].reshape(C)
    wqkvT = np.ascontiguousarray(
        np.concatenate([wq_r, wk_r, wv_r], axis=0).T).astype(BF16)
    # proj: input channels permuted to slot order
    perm_idx = (perm[:, None] * CH + np.arange(CH)[None, :]).reshape(-1)
    wpT = np.ascontiguousarray(proj_w.T[perm_idx]).astype(BF16)

    # per mask-group key compaction (mask quirk: group m uses mask[m])
    keep = [np.flatnonzero(~mask[m, 0]) for m in range(NMG)]
    n_max = max(max(len(kp) for kp in keep), 1)
    nkv = ((n_max + P - 1) // P) * P

    x_kv_all = []      # per batch: [NMG, C, nkv]
    for bb in range(B):
        xkv_b = np.zeros((NMG, C, nkv), dtype=BF16)
        for m in range(NMG):
            kp = keep[m]
            xkv_b[m, :, :len(kp)] = x[bb][:, kp]
        x_kv_all.append(xkv_b)

    pad = np.zeros((NMG, nkv, 1), dtype=np.float32)
    for m in range(NMG):
        pad[m, len(keep[m]):] = -1e9

    in_maps = []
    for core in range(8):
        bb, half = core // 2, core % 2
        ts = slice(half * TH, (half + 1) * TH)
        posb = pos[bb * H:(bb + 1) * H]        # [8, 64, 2048] true head order

        x_q = np.ascontiguousarray(x[bb][:, ts]).astype(BF16)
        x_res = np.ascontiguousarray(
            x[bb][:, ts] + proj_b[:, None]).astype(np.float32)
        pos_q = (posb[perm][:, :, ts].reshape(C, TH) + bq[:, None]).astype(BF16)

        pos_kv = np.zeros((NMG, P, nkv), dtype=BF16)
        posT = np.zeros((NMG, nkv, 130), dtype=np.float32)
        for m in range(NMG):
            kp = keep[m]
            nb = len(kp)
            for j, hh in enumerate((m, m + 4)):   # slots 2m, 2m+1
                sl = slice((2 * m + j) * CH, (2 * m + j + 1) * CH)
                pos_kv[m, j * CH:(j + 1) * CH, :nb] = (
                    posb[hh][:, kp] + bk[sl][:, None])
                posT[m, :nb, j * 65:j * 65 + CH] = (
                    posb[hh][:, kp].T + bv[sl][None, :])
                posT[m, :, j * 65 + CH] = 1.0

        in_maps.append({
            "x_q": x_q,
            "x_kv": x_kv_all[bb],
            "x_res": x_res,
            "gn_AB": np.ascontiguousarray(
                np.stack([A_all[bb], B_all[bb]], axis=1)).astype(np.float32),
            "pos_q": pos_q,
            "pos_kv": pos_kv,
            "posT_kv": posT.astype(BF16),
            "wqkvT": wqkvT,
            "wpT": wpT,
            "pad_bias": pad,
        })
    return nkv, in_maps


def kernel(**inputs):
    from concourse.bass_utils import run_bass_kernel_spmd

    nkv, in_maps = _prepare(inputs)
    if nkv not in _graph_cache:
        _graph_cache[nkv] = _build(nkv)
    nc = _graph_cache[nkv]

    res = run_bass_kernel_spmd(nc, in_maps, core_ids=list(range(8)))
    results = res.results

    out = np.empty((B, C, T), dtype=np.float32)
    for core in range(8):
        bb, half = core // 2, core % 2
        out[bb][:, half * TH:(half + 1) * TH] = results[core]["out"]
    return out



# revision 9
# speedup vs baseline: 1.3388x; 1.3388x over previous
"""Trainium2 Bass kernel for nn_AttentionBlock (sparse attention block).

Reference computation (B=4, C=512, T=2048, H=8 heads, 32 GN groups):
    xn  = GroupNorm(x) * gn_w + gn_b
    qkv = qkv_w @ xn + qkv_b            (1x1 conv)
    q,k,v = split(reshape(qkv, [B*H, 192, T])) ; each += pos
    S   = (q*s)^T (k*s),  s = ch^-0.25  => scale 1/8 on logits
    S[mask keys] = -1e9 ; P = softmax(S, axis=keys)
    h   = P @ v ; out = x + proj_w @ h + proj_b

Mask quirk (faithful to the reference): jnp.tile(mask,(H,1,1)) tiles
head-major, so attention row n = b*H + h uses mask[n % B] = mask[h % 4] —
every batch's head h is masked with mask[h mod 4], not its own batch mask.

Sharding: 8 cores = (batch b, query-half j).  Each core computes
out[b][:, j*1024:(j+1)*1024] completely; host concatenates.  No collectives.

Sparsity: host compacts the key axis per mask-group m = h%4 with
keep_m = ~mask[m] (about half of T), padded to a common multiple of 128.
Padded key rows get an exp-bias of -1e9 so they contribute exactly 0.

Head layout on device: slot order [0,4,1,5,2,6,3,7] so the two heads of a
mask-group (m, m+4) sit in one 128-partition pair; host reorders the qkv
weights / biases / pos / proj rows to match, so the device never permutes.

Device layout tricks: scores are computed transposed, S^T [keys, queries]:
  - the pad bias is per-partition and folds into the ACT exp for free,
  - the softmax denominator comes from an extra ones-column appended to V^T
    during the PV matmul (row 64 of the PV psum accumulates sum_s exp(S)).
GroupNorm statistics are folded on the host into a per-channel affine (A, B)
so the device applies xn = x*A + B with one tensor_scalar op per tile.
"""

import numpy as np
import ml_dtypes

B, C, T, H = 4, 512, 2048, 8
CH = C // H          # 64 channels per head
TH = T // 2          # 1024 query columns per core
P = 128
NUM_GROUPS = 32
GS = C // NUM_GROUPS  # 16 channels per group
EPS = 1e-5
BF16 = ml_dtypes.bfloat16
NMG = 4              # mask groups (= B); group m covers heads m and m+4
PERM = [0, 4, 1, 5, 2, 6, 3, 7]  # slot s holds true head PERM[s]

_graph_cache = {}


def _build(nkv):
    """Build the Bass graph for one core (SPMD: all 8 cores run this graph)."""
    import concourse.tile as tile
    from concourse import bacc, mybir

    f32 = mybir.dt.float32
    bf16 = mybir.dt.bfloat16
    AF = mybir.ActivationFunctionType
    OP = mybir.AluOpType

    sc_n = nkv // P  # number of 128-wide key chunks

    nc = bacc.Bacc("TRN2")

    # ---- DRAM parameters (per-core shards; host fills these) ----
    d_xq = nc.dram_tensor("x_q", [C, TH], bf16, kind="ExternalInput")
    d_xkv = nc.dram_tensor("x_kv", [NMG, C, nkv], bf16, kind="ExternalInput")
    d_xres = nc.dram_tensor("x_res", [C, TH], f32, kind="ExternalInput")
    d_AB = nc.dram_tensor("gn_AB", [C, 2], f32, kind="ExternalInput")
    d_posq = nc.dram_tensor("pos_q", [C, TH], bf16, kind="ExternalInput")
    d_poskv = nc.dram_tensor("pos_kv", [NMG, P, nkv], bf16, kind="ExternalInput")
    d_posT = nc.dram_tensor("posT_kv", [NMG, nkv, 130], bf16, kind="ExternalInput")
    d_wqkvT = nc.dram_tensor("wqkvT", [C, 3 * C], bf16, kind="ExternalInput")
    d_wpT = nc.dram_tensor("wpT", [C, C], bf16, kind="ExternalInput")
    d_pad = nc.dram_tensor("pad_bias", [NMG, nkv, 1], f32, kind="ExternalInput")
    d_out = nc.dram_tensor("out", [C, TH], f32, kind="ExternalOutput")

    with tile.TileContext(nc) as tc, \
         tc.tile_pool(name="persist", bufs=1) as pers, \
         tc.tile_pool(name="mm", bufs=2, space="PSUM") as mmp, \
         tc.tile_pool(name="opool", bufs=2, space="PSUM") as opl, \
         tc.tile_pool(name="exps", bufs=6) as epl, \
         tc.tile_pool(name="misc", bufs=2) as msc, \
         tc.tile_pool(name="nrm", bufs=1) as nrm:

        def ptile(shape, dt_, name):
            return pers.tile(shape, dt_, tag=name, name=name)

        # --- tiny exp to pull the ACT table load off the critical path ---
        warm_in = ptile([1, 1], f32, "warm_in")
        warm_out = ptile([1, 1], f32, "warm_out")
        nc.vector.memset(warm_in, 0.0)
        nc.scalar.activation(out=warm_out, in_=warm_in, func=AF.Exp)

        # --- persistent SBUF arrays (combined tiles = fewer, bigger DMAs) ---
        # xkv[m]: [128, 4 chan-blocks * nkv]; block i = channels [128i,128i+128)
        xkv = [ptile([P, 4 * nkv], bf16, f"xkv{m}") for m in range(NMG)]
        # vhat[m]: [128, sc_n * 130]; chunk s = columns [130s, 130s+130)
        vhat = [ptile([P, sc_n * 130], bf16, f"vhat{m}") for m in range(NMG)]
        pad_sb = [ptile([P, sc_n], f32, f"pad{m}") for m in range(NMG)]
        gnAB = ptile([P, 4 * 2], f32, "gnAB")       # block i = [A_i, B_i]
        wv = ptile([P, 4 * C], bf16, "wv")          # block i = v-rows of W^T
        wqk = ptile([P, 4 * 2 * C], bf16, "wqk")    # block i = [q|k]-rows
        xq = ptile([P, 4 * TH], bf16, "xq")
        posq = ptile([P, 4 * TH], bf16, "posq")
        poskv = [ptile([P, nkv], bf16, f"poskv{m}") for m in range(NMG)]
        xres = ptile([P, 4 * TH], f32, "xres")
        wp = ptile([CH, 8 * C], bf16, "wp")         # block cc = proj rows
        q_sb = [ptile([P, TH], bf16, f"q{i}") for i in range(4)]
        k_sb = [ptile([P, nkv], bf16, f"k{m}") for m in range(NMG)]
        # per-head-slot attention output (all at base partition 0)
        h_sb = [ptile([CH, TH], bf16, f"h{s}") for s in range(H)]

        # --- input DMAs, ordered by first use; HW-DGE prefetches ---
        def blk_dma(dst, src, nblk):
            # src [nblk*P, W] dram -> dst [P, nblk*W] sbuf (block i = rows
            # [i*P,(i+1)*P)); 3-dim APs on both sides, no dim grouping.
            w = src.shape[-1]
            nc.sync.dma_start(
                dst.rearrange("p (i w) -> p i w", i=nblk),
                src.rearrange("(i p) w -> p i w", p=dst.shape[0]))

        def dma_m_inputs(m):
            blk_dma(xkv[m], d_xkv[m], 4)
            blk_dma(vhat[m], d_posT[m], sc_n)
            blk_dma(pad_sb[m], d_pad[m], sc_n)

        blk_dma(gnAB, d_AB, 4)
        dma_m_inputs(0)
        blk_dma(wv, d_wqkvT[:, 2 * C:3 * C], 4)
        dma_m_inputs(1)
        blk_dma(wqk, d_wqkvT[:, 0:2 * C], 4)
        blk_dma(xq, d_xq, 4)
        blk_dma(posq, d_posq, 4)
        nc.sync.dma_start(poskv[0], d_poskv[0])
        dma_m_inputs(2)
        dma_m_inputs(3)
        for m in range(1, NMG):
            nc.sync.dma_start(poskv[m], d_poskv[m])
        blk_dma(wp, d_wpT, 8)
        blk_dma(xres, d_xres, 4)

        # --- GroupNorm as per-channel affine (host-computed A, B) ---
        def affine_kv(m):
            for i in range(4):
                sl = xkv[m][:, i * nkv:(i + 1) * nkv]
                nc.vector.tensor_scalar(
                    out=sl, in0=sl, scalar1=gnAB[:, 2 * i:2 * i + 1],
                    scalar2=gnAB[:, 2 * i + 1:2 * i + 2],
                    op0=OP.mult, op1=OP.add)

        def emit_v(m):
            # v^T for group m: (xn_kv_m)^T @ w_v[group m slots]^T
            for s in range(sc_n):
                pv = mmp.tile([P, P], f32, tag="mm", name=f"psv{m}_{s}")
                for i in range(4):
                    nc.tensor.matmul(
                        pv, xkv[m][:, i * nkv + s * P:i * nkv + (s + 1) * P],
                        wv[:, i * C + m * P:i * C + (m + 1) * P],
                        start=(i == 0), stop=(i == 3))
                vh_view = vhat[m][:, s * 130:(s + 1) * 130].rearrange(
                    "p (h c) -> p h c", c=65)[:, :, 0:CH]
                ps_view = pv.rearrange("p (h c) -> p h c", c=CH)
                nc.vector.tensor_tensor(vh_view, ps_view, vh_view, OP.add)

        nb_blocks = [(st, min(512, nkv - st)) for st in range(0, nkv, 512)]

        def emit_qk(m):
            # q channels (slot order) [128*m, 128*m+128)
            pq = mmp.tile([P, TH], f32, tag="mm", name=f"psq{m}")
            for tb in range(2):
                for i in range(4):
                    nc.tensor.matmul(
                        pq[:, tb * 512:(tb + 1) * 512],
                        wqk[:, 2 * i * C + m * P:2 * i * C + (m + 1) * P],
                        xq[:, i * TH + tb * 512:i * TH + (tb + 1) * 512],
                        start=(i == 0), stop=(i == 3))
            nc.vector.tensor_add(q_sb[m], pq, posq[:, m * TH:(m + 1) * TH])
            for bi, (st, w) in enumerate(nb_blocks):
                pk = mmp.tile([P, 512], f32, tag="mm", name=f"psk{m}_{bi}")
                for i in range(4):
                    nc.tensor.matmul(
                        pk[:, 0:w],
                        wqk[:, (2 * i + 1) * C + m * P:(2 * i + 1) * C + (m + 1) * P],
                        xkv[m][:, i * nkv + st:i * nkv + st + w],
                        start=(i == 0), stop=(i == 3))
                nc.vector.tensor_add(
                    k_sb[m][:, st:st + w], pk[:, 0:w],
                    poskv[m][:, st:st + w])

        def emit_attention(m):
            # pair m: head slot a=2m (partitions 0:64), b=2m+1 (64:128)
            o_a = opl.tile([65, TH], f32, tag="O", name=f"oa{m}")
            o_b = opl.tile([65, TH], f32, tag="O", name=f"ob{m}")
            for s in range(sc_n):
                sa = mmp.tile([P, TH], f32, tag="mm", name=f"sa{m}_{s}")
                sb = mmp.tile([P, TH], f32, tag="mm", name=f"sb{m}_{s}")
                for tb in range(2):
                    nc.tensor.matmul(
                        sa[:, tb * 512:(tb + 1) * 512],
                        k_sb[m][0:64, s * P:(s + 1) * P],
                        q_sb[m][0:64, tb * 512:(tb + 1) * 512],
                        start=True, stop=True)
                for tb in range(2):
                    nc.tensor.matmul(
                        sb[:, tb * 512:(tb + 1) * 512],
                        k_sb[m][64:128, s * P:(s + 1) * P],
                        q_sb[m][64:128, tb * 512:(tb + 1) * 512],
                        start=True, stop=True, tile_position=(64, 0))
                ex = epl.tile([P, 2 * TH], bf16, tag="expS", name=f"ex{m}_{s}")
                nc.scalar.activation(
                    out=ex[:, 0:TH], in_=sa, func=AF.Exp,
                    bias=pad_sb[m][:, s:s + 1], scale=0.125)
                nc.scalar.activation(
                    out=ex[:, TH:2 * TH], in_=sb, func=AF.Exp,
                    bias=pad_sb[m][:, s:s + 1], scale=0.125)
                for tb in range(2):
                    nc.tensor.matmul(
                        o_a[:, tb * 512:(tb + 1) * 512],
                        vhat[m][:, s * 130:s * 130 + 65],
                        ex[:, tb * 512:(tb + 1) * 512],
                        start=(s == 0), stop=(s == sc_n - 1))
                for tb in range(2):
                    nc.tensor.matmul(
                        o_b[:, tb * 512:(tb + 1) * 512],
                        vhat[m][:, s * 130 + 65:s * 130 + 130],
                        ex[:, TH + tb * 512:TH + (tb + 1) * 512],
                        start=(s == 0), stop=(s == sc_n - 1))
            return o_a, o_b

        def emit_normalize(m, o_a, o_b):
            # normalize: h = O[0:64] / l, l = O[64].  Entirely PE-free:
            # ACT copies the two denominator rows to partition 0 (Copy is in
            # every ACT table set, so no table reload), GPSIMD broadcasts
            # them over 64 partitions, and a 64-lane fast approx reciprocal
            # + DVE multiply finish the division.
            l_sb = nrm.tile([1, 2 * TH], f32, tag="lrow", name=f"l{m}")
            nc.scalar.activation(
                out=l_sb[:, 0:TH], in_=o_a[64:65, :], func=AF.Copy)
            nc.scalar.activation(
                out=l_sb[:, TH:2 * TH], in_=o_b[64:65, :], func=AF.Copy)
            lb = nrm.tile([CH, 2 * TH], f32, tag="lbc", name=f"lb{m}")
            nc.gpsimd.partition_broadcast(lb, l_sb)
            rc = nrm.tile([CH, 2 * TH], f32, tag="rcb", name=f"rcb{m}")
            nc.vector.reciprocal_approx_fast(out=rc, in_=lb)
            nc.vector.tensor_mul(
                h_sb[2 * m], o_a[0:64, :], rc[:, 0:TH])
            nc.vector.tensor_mul(
                h_sb[2 * m + 1], o_b[0:64, :], rc[:, TH:2 * TH])

        # --- pipeline: affine + V projection per mask group ASAP ---
        affine_kv(0)
        emit_v(0)
        affine_kv(1)
        emit_v(1)
        # xq affine (4 chan-blocks) before first emit_qk
        for i in range(4):
            sl = xq[:, i * TH:(i + 1) * TH]
            nc.vector.tensor_scalar(
                out=sl, in0=sl, scalar1=gnAB[:, 2 * i:2 * i + 1],
                scalar2=gnAB[:, 2 * i + 1:2 * i + 2], op0=OP.mult, op1=OP.add)
        affine_kv(2)
        emit_v(2)
        affine_kv(3)
        emit_v(3)

        # interleave: emit next group's qk before normalizing the
        # previous group so PE/ACT stay fed during the reciprocal
        pending = {}
        for m in range(NMG):
            emit_qk(m)
            if m - 1 in pending:
                emit_normalize(m - 1, *pending.pop(m - 1))
            pending[m] = emit_attention(m)
        emit_normalize(NMG - 1, *pending.pop(NMG - 1))

        # ---- proj + residual (contraction in 8 chunks of 64) ----
        for ci in range(4):
            pp = mmp.tile([P, TH], f32, tag="mm", name=f"pp{ci}")
            for tb in range(2):
                for cc in range(8):
                    nc.tensor.matmul(
                        pp[:, tb * 512:(tb + 1) * 512],
                        wp[:, cc * C + ci * P:cc * C + (ci + 1) * P],
                        h_sb[cc][:, tb * 512:(tb + 1) * 512],
                        start=(cc == 0), stop=(cc == 7))
                # fine-grained tail: add residual + DMA out per 512 cols
                ot = msc.tile([P, 512], f32, tag="out", name=f"ot{ci}_{tb}")
                nc.vector.tensor_add(
                    ot, pp[:, tb * 512:(tb + 1) * 512],
                    xres[:, ci * TH + tb * 512:ci * TH + (tb + 1) * 512])
                nc.sync.dma_start(
                    d_out[ci * P:(ci + 1) * P, tb * 512:(tb + 1) * 512], ot)

    nc.finalize()
    return nc


def _prepare(inputs):
    """Host-side shard preparation. Returns (nkv, in_maps)."""
    x = np.asarray(inputs["x"], dtype=np.float32)
    pos = np.asarray(inputs["pos"], dtype=np.float32)
    mask = np.asarray(inputs["mask"])
    gn_w = np.asarray(inputs["gn_w"], dtype=np.float32)
    gn_b = np.asarray(inputs["gn_b"], dtype=np.float32)
    qkv_w = np.asarray(inputs["qkv_w"], dtype=np.float32)
    qkv_b = np.asarray(inputs["qkv_b"], dtype=np.float32)
    proj_w = np.asarray(inputs["proj_w"], dtype=np.float32)
    proj_b = np.asarray(inputs["proj_b"], dtype=np.float32)

    # GroupNorm folded to per-channel affine per batch (stats over full T,
    # matching the reference exactly).
    xg = x.reshape(B, NUM_GROUPS, GS, T)
    mu = xg.mean(axis=(2, 3))
    var = xg.var(axis=(2, 3))
    rs = 1.0 / np.sqrt(var + EPS)
    rs_c = np.repeat(rs, GS, axis=1)
    mu_c = np.repeat(mu, GS, axis=1)
    A_all = rs_c * gn_w[None, :]
    B_all = gn_b[None, :] - mu_c * A_all

    # reorder qkv weights: reference splits rows as [h, (q|k|v), 64]; we
    # additionally permute heads into slot order PERM.
    perm = np.asarray(PERM)
    w3 = qkv_w.reshape(H, 3, CH, C)
    b3 = qkv_b.reshape(H, 3, CH)
    wq_r = w3[perm, 0].reshape(C, C)
    wk_r = w3[perm, 1].reshape(C, C)
    wv_r = w3[perm, 2].reshape(C, C)
    bq = b3[perm, 0].reshape(C)
    bk = b3[perm, 1].reshape(C)
    bv = b3[perm, 2].reshape(C)
    # device weight layout: wqkvT[:, 0:2C] = interleaved [q|k] per... actually
    # [q rows | k rows | v rows] transposed, same as before.
    wqkvT = np.ascontiguousarray(
        np.concatenate([wq_r, wk_r, wv_r], axis=0).T).astype(BF16)
    # proj: input channels permuted to slot order
    perm_idx = (perm[:, None] * CH + np.arange(CH)[None, :]).reshape(-1)
    wpT = np.ascontiguousarray(proj_w.T[perm_idx]).astype(BF16)

    # per mask-group key compaction (mask quirk: group m uses mask[m])
    keep = [np.flatnonzero(~mask[m, 0]) for m in range(NMG)]
    n_max = max(max(len(kp) for kp in keep), 1)
    nkv = ((n_max + P - 1) // P) * P

    x_kv_all = []      # per batch: [NMG, C, nkv]
    for bb in range(B):
        xkv_b = np.zeros((NMG, C, nkv), dtype=BF16)
        for m in range(NMG):
            kp = keep[m]
            xkv_b[m, :, :len(kp)] = x[bb][:, kp]
        x_kv_all.append(xkv_b)

    pad = np.zeros((NMG, nkv, 1), dtype=np.float32)
    for m in range(NMG):
        pad[m, len(keep[m]):] = -1e9

    in_maps = []
    for core in range(8):
        bb, half = core // 2, core % 2
        ts = slice(half * TH, (half + 1) * TH)
        posb = pos[bb * H:(bb + 1) * H]        # [8, 64, 2048] true head order

        x_q = np.ascontiguousarray(x[bb][:, ts]).astype(BF16)
        x_res = np.ascontiguousarray(
            x[bb][:, ts] + proj_b[:, None]).astype(np.float32)
        pos_q = (posb[perm][:, :, ts].reshape(C, TH) + bq[:, None]).astype(BF16)

        pos_kv = np.zeros((NMG, P, nkv), dtype=BF16)
        posT = np.zeros((NMG, nkv, 130), dtype=np.float32)
        for m in range(NMG):
            kp = keep[m]
            nb = len(kp)
            for j, hh in enumerate((m, m + 4)):   # slots 2m, 2m+1
                sl = slice((2 * m + j) * CH, (2 * m + j + 1) * CH)
                pos_kv[m, j * CH:(j + 1) * CH, :nb] = (
                    posb[hh][:, kp] + bk[sl][:, None])
                posT[m, :nb, j * 65:j * 65 + CH] = (
                    posb[hh][:, kp].T + bv[sl][None, :])
                posT[m, :, j * 65 + CH] = 1.0

        in_maps.append({
            "x_q": x_q,
            "x_kv": x_kv_all[bb],
            "x_res": x_res,
            "gn_AB": np.ascontiguousarray(
                np.stack([A_all[bb], B_all[bb]], axis=1)).astype(np.float32),
            "pos_q": pos_q,
            "pos_kv": pos_kv,
            "posT_kv": posT.astype(BF16),
            "wqkvT": wqkvT,
            "wpT": wpT,
            "pad_bias": pad,
        })
    return nkv, in_maps


def kernel(**inputs):
    from concourse.bass_utils import run_bass_kernel_spmd

    nkv, in_maps = _prepare(inputs)
    if nkv not in _graph_cache:
        _graph_cache[nkv] = _build(nkv)
    nc = _graph_cache[nkv]

    res = run_bass_kernel_spmd(nc, in_maps, core_ids=list(range(8)))
    results = res.results

    out = np.empty((B, C, T), dtype=np.float32)
    for core in range(8):
        bb, half = core // 2, core % 2
        out[bb][:, half * TH:(half + 1) * TH] = results[core]["out"]
    return out


# revision 19
# speedup vs baseline: 1.4489x; 1.0822x over previous
"""Trainium2 Bass kernel for nn_AttentionBlock (sparse attention block).

Reference computation (B=4, C=512, T=2048, H=8 heads, 32 GN groups):
    xn  = GroupNorm(x) * gn_w + gn_b
    qkv = qkv_w @ xn + qkv_b            (1x1 conv)
    q,k,v = split(reshape(qkv, [B*H, 192, T])) ; each += pos
    S   = (q*s)^T (k*s),  s = ch^-0.25  => scale 1/8 on logits
    S[mask keys] = -1e9 ; P = softmax(S, axis=keys)
    h   = P @ v ; out = x + proj_w @ h + proj_b

Mask quirk (faithful to the reference): jnp.tile(mask,(H,1,1)) tiles
head-major, so attention row n = b*H + h uses mask[n % B] = mask[h % 4] —
every batch's head h is masked with mask[h mod 4], not its own batch mask.

Sharding: 8 cores = (batch b, query-half j).  Each core computes
out[b][:, j*1024:(j+1)*1024] completely; host concatenates.  No collectives.

Sparsity: host compacts the key axis per mask-group m = h%4 with
keep_m = ~mask[m] (about half of T), padded to a common multiple of 128.
Padded key rows get an exp-bias of -1e9 so they contribute exactly 0.

Head layout on device: slot order [0,4,1,5,2,6,3,7] so the two heads of a
mask-group (m, m+4) sit in one 128-partition pair; host reorders the qkv
weights / biases / pos / proj rows to match, so the device never permutes.

Device layout tricks: scores are computed transposed, S^T [keys, queries]:
  - the pad bias is per-partition and folds into the ACT exp for free,
  - the softmax denominator comes from an extra ones-column appended to V^T
    during the PV matmul (row 64 of the PV psum accumulates sum_s exp(S)).
GroupNorm statistics are folded on the host into a per-channel affine (A, B)
so the device applies xn = x*A + B with one tensor_scalar op per tile.
"""

import numpy as np
import ml_dtypes

B, C, T, H = 4, 512, 2048, 8
CH = C // H          # 64 channels per head
TH = T // 2          # 1024 query columns per core
P = 128
NUM_GROUPS = 32
GS = C // NUM_GROUPS  # 16 channels per group
EPS = 1e-5
BF16 = ml_dtypes.bfloat16
NMG = 4              # mask groups (= B); group m covers heads m and m+4
PERM = [0, 4, 1, 5, 2, 6, 3, 7]  # slot s holds true head PERM[s]

_graph_cache = {}


def _build(nkv):
    """Build the Bass graph for one core (SPMD: all 8 cores run this graph)."""
    import concourse.tile as tile
    from concourse import bacc, mybir

    f32 = mybir.dt.float32
    bf16 = mybir.dt.bfloat16
    AF = mybir.ActivationFunctionType
    OP = mybir.AluOpType

    sc_n = nkv // P  # number of 128-wide key chunks

    nc = bacc.Bacc("TRN2")

    # ---- DRAM parameters (per-core shards; host fills these) ----
    d_xq = nc.dram_tensor("x_q", [C, TH], bf16, kind="ExternalInput")
    d_xkv = nc.dram_tensor("x_kv", [NMG, C, nkv], bf16, kind="ExternalInput")
    d_xres = nc.dram_tensor("x_res", [C, TH], f32, kind="ExternalInput")
    d_AB = nc.dram_tensor("gn_AB", [P, 8], f32, kind="ExternalInput")
    d_posq = nc.dram_tensor("pos_q", [C, TH], bf16, kind="ExternalInput")
    d_poskv = nc.dram_tensor("pos_kv", [NMG, P, nkv], bf16, kind="ExternalInput")
    d_posT = nc.dram_tensor("posT_kv", [NMG, nkv, 130], bf16, kind="ExternalInput")
    d_wqkvT = nc.dram_tensor("wqkvT", [C, 3 * C], bf16, kind="ExternalInput")
    d_wpT = nc.dram_tensor("wpT", [C, C], bf16, kind="ExternalInput")
    # pad/gnAB come pre-packed partition-major so their DMAs are one
    # contiguous run per partition (tiny-element DMAs cost µs to issue).
    d_pad = nc.dram_tensor("pad_bias", [NMG, P, nkv // P], f32,
                           kind="ExternalInput")
    d_out = nc.dram_tensor("out", [C, TH], f32, kind="ExternalOutput")

    with tile.TileContext(nc) as tc, \
         tc.tile_pool(name="persist", bufs=1) as pers, \
         tc.tile_pool(name="mm", bufs=2, space="PSUM") as mmp, \
         tc.tile_pool(name="opool", bufs=2, space="PSUM") as opl, \
         tc.tile_pool(name="exps", bufs=6) as epl, \
         tc.tile_pool(name="misc", bufs=2) as msc, \
         tc.tile_pool(name="nrm", bufs=1) as nrm:

        def ptile(shape, dt_, name):
            return pers.tile(shape, dt_, tag=name, name=name)

        # --- tiny exp to pull the ACT table load off the critical path ---
        warm_in = ptile([1, 1], f32, "warm_in")
        warm_out = ptile([1, 1], f32, "warm_out")
        nc.vector.memset(warm_in, 0.0)
        nc.scalar.activation(out=warm_out, in_=warm_in, func=AF.Exp)

        # --- persistent SBUF arrays (combined tiles = fewer, bigger DMAs) ---
        # xkv[m]: [128, 4 chan-blocks * nkv]; block i = channels [128i,128i+128)
        xkv = [ptile([P, 4 * nkv], bf16, f"xkv{m}") for m in range(NMG)]
        # vhat[m]: [128, sc_n * 130]; chunk s = columns [130s, 130s+130)
        vhat = [ptile([P, sc_n * 130], bf16, f"vhat{m}") for m in range(NMG)]
        pad_sb = [ptile([P, sc_n], f32, f"pad{m}") for m in range(NMG)]
        gnAB = ptile([P, 4 * 2], f32, "gnAB")       # block i = [A_i, B_i]
        wv = ptile([P, 4 * C], bf16, "wv")          # block i = v-rows of W^T
        wqk = ptile([P, 4 * 2 * C], bf16, "wqk")    # block i = [q|k]-rows
        xq = ptile([P, 4 * TH], bf16, "xq")
        posq = ptile([P, 4 * TH], bf16, "posq")
        poskv = [ptile([P, nkv], bf16, f"poskv{m}") for m in range(NMG)]
        xres = ptile([P, 4 * TH], f32, "xres")
        wp = ptile([CH, 8 * C], bf16, "wp")         # block cc = proj rows
        q_sb = [ptile([P, TH], bf16, f"q{i}") for i in range(4)]
        k_sb = [ptile([P, nkv], bf16, f"k{m}") for m in range(NMG)]
        # per-head-slot attention output (all at base partition 0)
        h_sb = [ptile([CH, TH], bf16, f"h{s}") for s in range(H)]

        # --- input DMAs, ordered by first use; round-robin the issuing
        # engine (each dma_start costs ~0.7µs of issue time on its queue).
        _eng = [nc.sync, nc.scalar]
        _ei = [0]

        def dma(dst, src):
            _eng[_ei[0] % 2].dma_start(dst, src)
            _ei[0] += 1

        def blk_dma(dst, src, nblk):
            # src [nblk*P, W] dram -> dst [P, nblk*W] sbuf, one DMA per
            # row-block i so the issues spread across engines/queues.
            w = src.shape[-1]
            pp_ = dst.shape[0]
            for i in range(nblk):
                dma(dst[:, i * w:(i + 1) * w], src[i * pp_:(i + 1) * pp_, :])

        def dma_m_inputs(m):
            blk_dma(xkv[m], d_xkv[m, :, :], 4)
            _eng[_ei[0] % 2].dma_start(
                vhat[m].rearrange("p (s w) -> p s w", s=sc_n),
                d_posT[m].rearrange("(s p) w -> p s w", p=P))
            _ei[0] += 1
            dma(pad_sb[m], d_pad[m, :, :])

        dma(gnAB, d_AB[:, :])
        blk_dma(wv, d_wqkvT[:, 2 * C:3 * C], 4)
        dma_m_inputs(0)
        dma_m_inputs(1)
        blk_dma(wqk, d_wqkvT[:, 0:2 * C], 4)
        blk_dma(xq, d_xq[:, :], 4)
        blk_dma(posq, d_posq[:, :], 4)
        dma(poskv[0], d_poskv[0, :, :])
        dma_m_inputs(2)
        dma_m_inputs(3)
        for m in range(1, NMG):
            dma(poskv[m], d_poskv[m, :, :])
        blk_dma(wp, d_wpT[:, :], 8)
        blk_dma(xres, d_xres[:, :], 4)

        # --- GroupNorm as per-channel affine (host-computed A, B) ---
        def affine_kv(m):
            for i in range(4):
                sl = xkv[m][:, i * nkv:(i + 1) * nkv]
                nc.vector.tensor_scalar(
                    out=sl, in0=sl, scalar1=gnAB[:, 2 * i:2 * i + 1],
                    scalar2=gnAB[:, 2 * i + 1:2 * i + 2],
                    op0=OP.mult, op1=OP.add)

        def emit_v(m):
            # v^T for group m: (xn_kv_m)^T @ w_v[group m slots]^T
            for s in range(sc_n):
                pv = mmp.tile([P, P], f32, tag="mm", name=f"psv{m}_{s}")
                for i in range(4):
                    nc.tensor.matmul(
                        pv, xkv[m][:, i * nkv + s * P:i * nkv + (s + 1) * P],
                        wv[:, i * C + m * P:i * C + (m + 1) * P],
                        start=(i == 0), stop=(i == 3))
                vh_view = vhat[m][:, s * 130:(s + 1) * 130].rearrange(
                    "p (h c) -> p h c", c=65)[:, :, 0:CH]
                ps_view = pv.rearrange("p (h c) -> p h c", c=CH)
                nc.vector.tensor_tensor(vh_view, ps_view, vh_view, OP.add)

        nb_blocks = [(st, min(512, nkv - st)) for st in range(0, nkv, 512)]

        def emit_qk(m):
            # q channels (slot order) [128*m, 128*m+128)
            pq = mmp.tile([P, TH], f32, tag="mm", name=f"psq{m}")
            for tb in range(2):
                for i in range(4):
                    nc.tensor.matmul(
                        pq[:, tb * 512:(tb + 1) * 512],
                        wqk[:, 2 * i * C + m * P:2 * i * C + (m + 1) * P],
                        xq[:, i * TH + tb * 512:i * TH + (tb + 1) * 512],
                        start=(i == 0), stop=(i == 3))
            nc.vector.tensor_add(q_sb[m], pq, posq[:, m * TH:(m + 1) * TH])
            for bi, (st, w) in enumerate(nb_blocks):
                pk = mmp.tile([P, 512], f32, tag="mm", name=f"psk{m}_{bi}")
                for i in range(4):
                    nc.tensor.matmul(
                        pk[:, 0:w],
                        wqk[:, (2 * i + 1) * C + m * P:(2 * i + 1) * C + (m + 1) * P],
                        xkv[m][:, i * nkv + st:i * nkv + st + w],
                        start=(i == 0), stop=(i == 3))
                nc.vector.tensor_add(
                    k_sb[m][:, st:st + w], pk[:, 0:w],
                    poskv[m][:, st:st + w])

        def emit_attention(m):
            # pair m: head slot a=2m (partitions 0:64), b=2m+1 (64:128)
            o_a = opl.tile([65, TH], f32, tag="O", name=f"oa{m}")
            o_b = opl.tile([65, TH], f32, tag="O", name=f"ob{m}")
            for s in range(sc_n):
                sa = mmp.tile([P, TH], f32, tag="mm", name=f"sa{m}_{s}")
                sb = mmp.tile([P, TH], f32, tag="mm", name=f"sb{m}_{s}")
                for tb in range(2):
                    nc.tensor.matmul(
                        sa[:, tb * 512:(tb + 1) * 512],
                        k_sb[m][0:64, s * P:(s + 1) * P],
                        q_sb[m][0:64, tb * 512:(tb + 1) * 512],
                        start=True, stop=True)
                for tb in range(2):
                    nc.tensor.matmul(
                        sb[:, tb * 512:(tb + 1) * 512],
                        k_sb[m][64:128, s * P:(s + 1) * P],
                        q_sb[m][64:128, tb * 512:(tb + 1) * 512],
                        start=True, stop=True, tile_position=(64, 0))
                ex = epl.tile([P, 2 * TH], bf16, tag="expS", name=f"ex{m}_{s}")
                nc.scalar.activation(
                    out=ex[:, 0:TH], in_=sa, func=AF.Exp,
                    bias=pad_sb[m][:, s:s + 1], scale=0.125)
                nc.scalar.activation(
                    out=ex[:, TH:2 * TH], in_=sb, func=AF.Exp,
                    bias=pad_sb[m][:, s:s + 1], scale=0.125)
                for tb in range(2):
                    nc.tensor.matmul(
                        o_a[:, tb * 512:(tb + 1) * 512],
                        vhat[m][:, s * 130:s * 130 + 65],
                        ex[:, tb * 512:(tb + 1) * 512],
                        start=(s == 0), stop=(s == sc_n - 1))
                for tb in range(2):
                    nc.tensor.matmul(
                        o_b[:, tb * 512:(tb + 1) * 512],
                        vhat[m][:, s * 130 + 65:s * 130 + 130],
                        ex[:, TH + tb * 512:TH + (tb + 1) * 512],
                        start=(s == 0), stop=(s == sc_n - 1))
            return o_a, o_b

        def emit_normalize(m, o_a, o_b):
            # normalize: h = O[0:64] / l, l = O[64].  Entirely PE-free:
            # ACT copies the two denominator rows to partition 0 (Copy is in
            # every ACT table set, so no table reload), GPSIMD broadcasts
            # them over 64 partitions, and a 64-lane fast approx reciprocal
            # + DVE multiply finish the division.
            l_sb = nrm.tile([1, 2 * TH], f32, tag="lrow", name=f"l{m}")
            nc.scalar.activation(
                out=l_sb[:, 0:TH], in_=o_a[64:65, :], func=AF.Copy)
            nc.scalar.activation(
                out=l_sb[:, TH:2 * TH], in_=o_b[64:65, :], func=AF.Copy)
            lb = nrm.tile([CH, 2 * TH], f32, tag="lbc", name=f"lb{m}")
            nc.gpsimd.partition_broadcast(lb, l_sb)
            rc = nrm.tile([CH, 2 * TH], f32, tag="rcb", name=f"rcb{m}")
            nc.vector.reciprocal_approx_fast(out=rc, in_=lb)
            nc.vector.tensor_mul(
                h_sb[2 * m], o_a[0:64, :], rc[:, 0:TH])
            nc.vector.tensor_mul(
                h_sb[2 * m + 1], o_b[0:64, :], rc[:, TH:2 * TH])

        # --- pipeline: affine + V projection per mask group ASAP ---
        affine_kv(0)
        emit_v(0)
        affine_kv(1)
        emit_v(1)
        # xq affine (4 chan-blocks) before first emit_qk
        for i in range(4):
            sl = xq[:, i * TH:(i + 1) * TH]
            nc.vector.tensor_scalar(
                out=sl, in0=sl, scalar1=gnAB[:, 2 * i:2 * i + 1],
                scalar2=gnAB[:, 2 * i + 1:2 * i + 2], op0=OP.mult, op1=OP.add)
        affine_kv(2)
        emit_v(2)
        affine_kv(3)
        emit_v(3)

        # interleave: emit next group's qk before normalizing the
        # previous group so PE/ACT stay fed during the reciprocal
        pending = {}
        for m in range(NMG):
            emit_qk(m)
            if m - 1 in pending:
                emit_normalize(m - 1, *pending.pop(m - 1))
            pending[m] = emit_attention(m)

        # ---- proj + residual (contraction in 8 chunks of 64) ----
        # Interleave with the last group's normalize: emit the first 6
        # accumulation steps (head slots 0..5, groups 0-2) for two output
        # chunks FIRST so the PE chews on them while the (PE-free)
        # normalize(3) chain produces h_sb[6], h_sb[7].
        pp_t = {}

        def proj_acc(ci, ccs, stop_cc):
            if ci not in pp_t:
                pp_t[ci] = mmp.tile([P, TH], f32, tag="mm", name=f"pp{ci}")
            pp = pp_t[ci]
            for tb in range(2):
                for cc in ccs:
                    nc.tensor.matmul(
                        pp[:, tb * 512:(tb + 1) * 512],
                        wp[:, cc * C + ci * P:cc * C + (ci + 1) * P],
                        h_sb[cc][:, tb * 512:(tb + 1) * 512],
                        start=(cc == 0), stop=(cc == stop_cc))

        def proj_out(ci):
            pp = pp_t[ci]
            for tb in range(2):
                ot = msc.tile([P, 512], f32, tag="out", name=f"ot{ci}_{tb}")
                nc.vector.tensor_add(
                    ot, pp[:, tb * 512:(tb + 1) * 512],
                    xres[:, ci * TH + tb * 512:ci * TH + (tb + 1) * 512])
                nc.sync.dma_start(
                    d_out[ci * P:(ci + 1) * P, tb * 512:(tb + 1) * 512], ot)

        proj_acc(0, range(6), 7)
        proj_acc(1, range(6), 7)
        emit_normalize(NMG - 1, *pending.pop(NMG - 1))
        for ci in (0, 1):
            proj_acc(ci, (6, 7), 7)
            proj_out(ci)
        for ci in (2, 3):
            proj_acc(ci, range(8), 7)
            proj_out(ci)

    nc.finalize()
    return nc


def _prepare(inputs):
    """Host-side shard preparation. Returns (nkv, in_maps)."""
    x = np.asarray(inputs["x"], dtype=np.float32)
    pos = np.asarray(inputs["pos"], dtype=np.float32)
    mask = np.asarray(inputs["mask"])
    gn_w = np.asarray(inputs["gn_w"], dtype=np.float32)
    gn_b = np.asarray(inputs["gn_b"], dtype=np.float32)
    qkv_w = np.asarray(inputs["qkv_w"], dtype=np.float32)
    qkv_b = np.asarray(inputs["qkv_b"], dtype=np.float32)
    proj_w = np.asarray(inputs["proj_w"], dtype=np.float32)
    proj_b = np.asarray(inputs["proj_b"], dtype=np.float32)

    # GroupNorm folded to per-channel affine per batch (stats over full T,
    # matching the reference exactly).
    xg = x.reshape(B, NUM_GROUPS, GS, T)
    mu = xg.mean(axis=(2, 3))
    var = xg.var(axis=(2, 3))
    rs = 1.0 / np.sqrt(var + EPS)
    rs_c = np.repeat(rs, GS, axis=1)
    mu_c = np.repeat(mu, GS, axis=1)
    A_all = rs_c * gn_w[None, :]
    B_all = gn_b[None, :] - mu_c * A_all

    # reorder qkv weights: reference splits rows as [h, (q|k|v), 64]; we
    # additionally permute heads into slot order PERM.
    perm = np.asarray(PERM)
    w3 = qkv_w.reshape(H, 3, CH, C)
    b3 = qkv_b.reshape(H, 3, CH)
    wq_r = w3[perm, 0].reshape(C, C)
    wk_r = w3[perm, 1].reshape(C, C)
    wv_r = w3[perm, 2].reshape(C, C)
    bq = b3[perm, 0].reshape(C)
    bk = b3[perm, 1].reshape(C)
    bv = b3[perm, 2].reshape(C)
    # device weight layout: wqkvT[:, 0:2C] = interleaved [q|k] per... actually
    # [q rows | k rows | v rows] transposed, same as before.
    wqkvT = np.ascontiguousarray(
        np.concatenate([wq_r, wk_r, wv_r], axis=0).T).astype(BF16)
    # proj: input channels permuted to slot order
    perm_idx = (perm[:, None] * CH + np.arange(CH)[None, :]).reshape(-1)
    wpT = np.ascontiguousarray(proj_w.T[perm_idx]).astype(BF16)

    # per mask-group key compaction (mask quirk: group m uses mask[m])
    keep = [np.flatnonzero(~mask[m, 0]) for m in range(NMG)]
    n_max = max(max(len(kp) for kp in keep), 1)
    nkv = ((n_max + P - 1) // P) * P

    x_kv_all = []      # per batch: [NMG, C, nkv]
    for bb in range(B):
        xkv_b = np.zeros((NMG, C, nkv), dtype=BF16)
        for m in range(NMG):
            kp = keep[m]
            xkv_b[m, :, :len(kp)] = x[bb][:, kp]
        x_kv_all.append(xkv_b)

    # packed partition-major: pad[m, p, s] = bias for key s*128 + p
    pad = np.zeros((NMG, nkv), dtype=np.float32)
    for m in range(NMG):
        pad[m, len(keep[m]):] = -1e9
    pad = np.ascontiguousarray(
        pad.reshape(NMG, nkv // P, P).transpose(0, 2, 1))

    in_maps = []
    for core in range(8):
        bb, half = core // 2, core % 2
        ts = slice(half * TH, (half + 1) * TH)
        posb = pos[bb * H:(bb + 1) * H]        # [8, 64, 2048] true head order

        x_q = np.ascontiguousarray(x[bb][:, ts]).astype(BF16)
        x_res = np.ascontiguousarray(
            x[bb][:, ts] + proj_b[:, None]).astype(np.float32)
        pos_q = (posb[perm][:, :, ts].reshape(C, TH) + bq[:, None]).astype(BF16)

        pos_kv = np.zeros((NMG, P, nkv), dtype=BF16)
        posT = np.zeros((NMG, nkv, 130), dtype=np.float32)
        for m in range(NMG):
            kp = keep[m]
            nb = len(kp)
            for j, hh in enumerate((m, m + 4)):   # slots 2m, 2m+1
                sl = slice((2 * m + j) * CH, (2 * m + j + 1) * CH)
                pos_kv[m, j * CH:(j + 1) * CH, :nb] = (
                    posb[hh][:, kp] + bk[sl][:, None])
                posT[m, :nb, j * 65:j * 65 + CH] = (
                    posb[hh][:, kp].T + bv[sl][None, :])
                posT[m, :, j * 65 + CH] = 1.0

        in_maps.append({
            "x_q": x_q,
            "x_kv": x_kv_all[bb],
            "x_res": x_res,
            "gn_AB": np.ascontiguousarray(
                np.stack([A_all[bb], B_all[bb]], axis=1).reshape(
                    4, P, 2).transpose(1, 0, 2).reshape(P, 8)
            ).astype(np.float32),
            "pos_q": pos_q,
            "pos_kv": pos_kv,
            "posT_kv": posT.astype(BF16),
            "wqkvT": wqkvT,
            "wpT": wpT,
            "pad_bias": pad,
        })
    return nkv, in_maps


def kernel(**inputs):
    from concourse.bass_utils import run_bass_kernel_spmd

    nkv, in_maps = _prepare(inputs)
    if nkv not in _graph_cache:
        _graph_cache[nkv] = _build(nkv)
    nc = _graph_cache[nkv]

    res = run_bass_kernel_spmd(nc, in_maps, core_ids=list(range(8)))
    results = res.results

    out = np.empty((B, C, T), dtype=np.float32)
    for core in range(8):
        bb, half = core // 2, core % 2
        out[bb][:, half * TH:(half + 1) * TH] = results[core]["out"]
    return out
